# revision 30
# baseline (speedup 1.0000x reference)
"""Trainium2 Bass kernel for nn_LstmRNN: 8-core tensor-parallel LSTM.

Strategy (8 NeuronCores, SPMD):
  - Tensor-parallel split of the 4H gate dimension: core j owns hidden-state
    slice j (128 of 1024 dims) and the matching 512 columns of W_ih / W_hh
    (gate order permuted host-side to [i, f, o, g] so the sigmoid covers
    contiguous blocks of the transposed gates).
  - Phase 1: x_gates^T = W_ih_local^T @ xs^T, bf16 matmuls, fp32 PSUM, bias
    fused into the eviction. Blocks 0-1 run as a prologue; blocks 2-7 are
    emitted as per-step PE filler (2 matmuls/step) inside the recurrence's
    exchange windows, with evictions on ACT pinned after the step's h-write
    so they never land inside the sigmoid chain.
  - Phase 2: 128 serial steps. Per step: inject x_gates into PSUM via an
    identity matmul (overlaps the exchange window), run the 8 K-chunk
    matmuls of h^T @ W_hh gate-major (g first, o last — measured best on HW
    vs chunk-major-in-arrival-order variants), so the c-chain EW (ACT
    sigmoid + DVE mul/add, relu kept on DVE to avoid engine hops) overlaps
    the later gates' groups. The core's h^T chunk [128, 64] bf16 is
    AllGathered across the 8 cores (SBUF -> DRAM bounce -> ncfw collective
    -> DRAM -> SBUF readback as 4 two-chunk DMAs split across the SP and ACT
    HWDGE engines, since each dma_start costs ~660 ns of engine issue time).
  - Phase 3: out = h @ W_o + b_o, bias via a K=1 ones-row matmul.

All matmul operands are bf16; state c and elementwise math stay fp32.
Validated vs the fp32 reference: absmax-relative error ~2.3e-3 (gate 2e-2).

Measured (neuron-profile, traced): ~2.14 ms; per step ~16 us = ~7.1 us ncfw
AllGather occupancy + ~2.4 us bounce-out/trigger + ~1.4 us readback issues
+ ~2 us matmuls + ~1.6 us EW chain + slack. The ncfw collective is the
floor: measured AllGather latency 13.2 us / throughput ~8.7 us even 8-deep
pipelined, so per-step-exchange designs cannot go below ~12 us/step here.

Exchange alternatives measured on HW and rejected:
  - remote_dma_broadcast, one instr with 7 or 8 dests: NRT_EXEC_UNIT_
    UNRECOVERABLE device fault (reproducible, multi-dest is broken here;
    self-send exclusion does not help; 3+4-dest split also faults).
  - 7 singleton broadcasts/step ("rdma7"): stable but ~100 us/step — each
    broadcast writes full 64-descriptor frames on all 16 DMA engine lanes
    (dummies included) and deliveries serialize (~88% of packets are 4-byte
    dummies, engines sustain ~1 desc per 70-100 ns).
  - remote_dma / remote_dma_fused (no dummy lanes, would be ~1-2 us): needs
    the absolute routing id of the local chip, which is not discoverable
    from an SPMD program or the axon client, so it cannot be used here.
  - G batch-group staggered recurrences to hide collective latency: dead on
    arrival since ncfw AllGather THROUGHPUT is ~8.7 us (G groups mean G
    collectives per step).
"""

import sys

for _p in ("/opt/trn_rl_repo",):
    if _p not in sys.path:
        sys.path.insert(0, _p)

import numpy as np
import ml_dtypes

import concourse.bass as bass
import concourse.mybir as mybir
import concourse.tile as tile
from concourse import bacc
from concourse import bass_utils
from concourse.bass import _add_dep_helper

BF16 = ml_dtypes.bfloat16

B, T, I, H, O = 64, 128, 512, 1024, 512
NCORES = 8
HSL = H // NCORES          # 128 hidden dims per core
GCOLS = 4 * HSL            # 512 gate columns per core (i,f,o,g x 128)
NB = T * B // 512          # phase-1 512-wide col-chunks (16)

F32 = mybir.dt.float32
BF = mybir.dt.bfloat16
AF = mybir.ActivationFunctionType
ALU = mybir.AluOpType


def build_program(t_steps: int = T, exchange: str = "rdma", interleave: bool = True):
    """exchange: 'rdma' = SBUF->SBUF remote_dma_broadcast; 'cc' = ncfw AllGather.

    interleave: emit phase-1 blocks 2..7 as per-step PE filler inside the
    recurrence (2 matmuls/step), so the x_gates precompute hides in the
    per-step exchange windows instead of running serially up front.
    """
    nc = bacc.Bacc(
        "TRN2",
        target_bir_lowering=False,
        debug=False,
        num_devices=NCORES,
    )

    xs_t = nc.dram_tensor("xs_t", [I, T * B], BF, kind="ExternalInput")
    wih = nc.dram_tensor("wih", [I, GCOLS], BF, kind="ExternalInput")
    whh = nc.dram_tensor("whh", [H, GCOLS], BF, kind="ExternalInput")
    bias = nc.dram_tensor("bias", [HSL, 4], F32, kind="ExternalInput")
    wo = nc.dram_tensor("wo", [H, O], BF, kind="ExternalInput")
    bo = nc.dram_tensor("bo", [1, O], BF, kind="ExternalInput")
    ident = nc.dram_tensor("ident", [128, 128], BF, kind="ExternalInput")
    ones = nc.dram_tensor("ones", [1, B], BF, kind="ExternalInput")
    out = nc.dram_tensor("out", [B, O], F32, kind="ExternalOutput")

    with tile.TileContext(nc) as tc:
        with (
            tc.tile_pool(name="consts", bufs=1) as consts,
            tc.tile_pool(name="xg", bufs=1) as xgp,
            tc.tile_pool(name="xsp", bufs=2) as xsp,
            tc.tile_pool(name="psum", bufs=2, space="PSUM") as psp,
            tc.tile_pool(name="ew", bufs=2) as ew,
            tc.tile_pool(name="hall", bufs=2) as hallp,
            tc.tile_pool(name="dram", bufs=2, space="DRAM") as dram,
        ):
            # ---- constants into SBUF ----
            wih_sb = consts.tile([128, 4, GCOLS], BF)
            nc.sync.dma_start(wih_sb[:], wih.rearrange("(k p) c -> p k c", p=128))
            whh_sb = consts.tile([128, 8, GCOLS], BF)
            nc.sync.dma_start(whh_sb[:], whh.rearrange("(k p) c -> p k c", p=128))
            bias_sb = consts.tile([HSL, 4], F32)
            nc.sync.dma_start(bias_sb[:], bias[:, :])
            id_sb = consts.tile([128, 128], BF)
            nc.sync.dma_start(id_sb[:], ident[:, :])
            ones_sb = consts.tile([1, B], BF)
            nc.sync.dma_start(ones_sb[:], ones[:, :])
            wo_sb = consts.tile([128, 8, O], BF)
            nc.sync.dma_start(wo_sb[:], wo.rearrange("(k p) c -> p k c", p=128))
            bo_sb = consts.tile([1, O], BF)
            nc.sync.dma_start(bo_sb[:], bo[:, :])

            # x_gates^T per gate chunk: [128 gate-dims, t*64+b]
            xg = [
                xgp.tile([128, T * B], BF, tag=f"xg{g}", name=f"xg{g}")
                for g in range(4)
            ]

            # ---- phase 1: x_gates^T = W_ih_local^T @ xs^T (+ bias) ----
            # PSUM budget: 4 tags x bufs=2 = 8 banks, shared with phase 2/3.
            ptags = ["pA", "pB", "pC", "pD"]
            NBLK = T * B // 1024  # 8 panels of 1024 cols (16 timesteps each)
            xs_r = xs_t.rearrange("(k p) n -> p k n", p=128)

            def load_panel(blk):
                panel = xsp.tile([128, 4, 1024], BF, tag="panel")
                nc.sync.dma_start(
                    panel[:], xs_r[:, :, blk * 1024 : (blk + 1) * 1024]
                )
                return panel

            def phase1_block(blk, panel):
                for g in range(4):
                    for sub in range(2):
                        ps = psp.tile(
                            [128, 512], F32, tag=ptags[g], name=f"ps1_{g}"
                        )
                        for k in range(4):
                            nc.tensor.matmul(
                                ps[:],
                                wih_sb[:, k, g * 128 : (g + 1) * 128],
                                panel[:, k, sub * 512 : (sub + 1) * 512],
                                start=(k == 0),
                                stop=(k == 3),
                            )
                        col0 = blk * 1024 + sub * 512
                        nc.vector.tensor_scalar(
                            xg[g][:, col0 : col0 + 512],
                            ps[:],
                            bias_sb[:, g : g + 1],
                            None,
                            ALU.add,
                        )

            n_prologue = 2 if interleave else NBLK
            for blk in range(n_prologue):
                phase1_block(blk, load_panel(blk))

            # filler evictions are pinned after the current step's h-write so
            # the ACT engine never runs them inside the sigmoid chain.
            evict_pin = [None]

            def filler_gen():
                # Blocks 2..7 as 96 half-units (2 matmuls each), one per
                # step. Yields the last PE matmul of each half-unit so the
                # caller can pin PE program order; evictions go to ACT
                # (Identity + per-partition bias) since DVE runs the cell EW.
                next_panel = load_panel(2)
                for blk in range(2, NBLK):
                    panel = next_panel
                    if blk + 1 < NBLK:
                        next_panel = load_panel(blk + 1)
                    for g in range(4):
                        for sub in range(2):
                            ps = psp.tile(
                                [128, 512], F32, tag=ptags[g], name=f"fps{blk}_{g}{sub}"
                            )
                            for k in range(2):
                                mm = nc.tensor.matmul(
                                    ps[:],
                                    wih_sb[:, k, g * 128 : (g + 1) * 128],
                                    panel[:, k, sub * 512 : (sub + 1) * 512],
                                    start=(k == 0),
                                    stop=False,
                                )
                            yield mm
                            for k in range(2, 4):
                                mm = nc.tensor.matmul(
                                    ps[:],
                                    wih_sb[:, k, g * 128 : (g + 1) * 128],
                                    panel[:, k, sub * 512 : (sub + 1) * 512],
                                    start=False,
                                    stop=(k == 3),
                                )
                            col0 = blk * 1024 + sub * 512
                            ev = nc.scalar.activation(
                                xg[g][:, col0 : col0 + 512],
                                ps[:],
                                AF.Identity,
                                bias=bias_sb[:, g : g + 1],
                            )
                            if evict_pin[0] is not None:
                                _add_dep_helper(
                                    ev.ins,
                                    evict_pin[0].ins,
                                    False,
                                    "evict after h-write",
                                )
                            yield mm

            fgen = filler_gen() if interleave else None

            # ---- phase 2: recurrence ----
            h_all = hallp.tile([128, 8 * B], BF, tag="hall")
            nc.vector.memset(h_all[:], 0.0)
            c_prev = ew.tile([128, B], F32, tag="c")
            nc.vector.memset(c_prev[:], 0.0)

            # rdest slot lists per broadcast instruction. Slot k carries
            # Δtpb=k; slots 4-7 are the D2D-capable lanes (cross-die).
            _D = {
                # one 7-dest broadcast (multi-dest: faults the device)
                "rdma": [[None] + [(0, k) for k in range(1, NCORES)]],
                # same-die dests and cross-die dests in separate broadcasts
                "rdma2": [
                    [None, (0, 1), (0, 2), (0, 3), None, None, None, None],
                    [None, None, None, None, (0, 4), (0, 5), (0, 6), (0, 7)],
                ],
                # one broadcast per destination (stable but slow per prior session)
                "rdma7": [
                    [
                        (0, k) if s == k else None
                        for s in range(NCORES)
                    ]
                    for k in range(1, NCORES)
                ],
            }
            rdest_groups = _D.get(exchange)
            if rdest_groups is not None:
                exchange = "rdma"

            if exchange == "rdma":
                # remote-DMA all-gather: per step, ONE 8-destination broadcast
                # (relative peers own^k on engine pair k/k+8; self included).
                # Data lands at the sender's absolute slot via ts(partition_id).
                # Each receiver gets +2 per sender -> +16 per step.
                #
                # Soundness of the shared counter: the remote sem alternates
                # by step parity. While a core sits at its step-t wait, no
                # peer can have issued a step-t+1 send (it would need this
                # core's step-t chunk), so only sends of steps <= t exist,
                # and step-t sends hit the OTHER parity sem. Per-engine FIFO
                # delivery makes each sender's counted prefix complete, so
                # rsem[(t-1)%2] == 16*ceil(t/2) implies every step-(t-1)
                # chunk has landed.
                #
                # Tile's single-core scheduling sim cannot see the remote
                # increments, so waits are emitted with value 0 (trivially
                # true for the scheduler), pinned in PE program order via
                # nosync deps, and the real thresholds are patched in after
                # scheduling (deferred_waits).
                rsems = [nc.alloc_semaphore(f"rdma_rsem{p}") for p in range(2)]
                lsem = nc.alloc_semaphore("rdma_lsem")
                pid_sv = nc.gpsimd.partition_id()
                pid_dve = nc.vector.partition_id()
                prev_hw = None
            deferred_waits = []
            prev_mm = None

            for t in range(t_steps):
                step_waits = []
                if exchange == "rdma" and t > 0:
                    # gate this step's matmuls on all 7 peer chunk arrivals
                    # (own chunk is written locally; each peer bumps +2)
                    w = nc.tensor.wait_ge(rsems[(t - 1) % 2], 0)
                    deferred_waits.append((w, 14 * ((t + 1) // 2)))
                    step_waits.append(w)
                    if prev_mm is not None:
                        _add_dep_helper(
                            w.ins,
                            prev_mm.ins,
                            False,
                            "rdma wait after prev step",
                        )
                # one PSUM bank per gate so EW reads overlap later gates' MMs
                pst = [
                    psp.tile([128, B], F32, tag=ptags[g], name=f"ps2_{g}")
                    for g in range(4)
                ]
                # gate col order: 0=i, 1=f, 2=o, 3=g. The x_gates injects
                # don't need h_all, so they are emitted first and overlap the
                # exchange; W-groups run g first / o last so the sigmoid/mul
                # chain overlaps the remaining MM groups.
                for g in (3, 0, 1, 2):
                    nc.tensor.matmul(
                        pst[g][:],
                        id_sb[:],
                        xg[g][:, t * B : (t + 1) * B],
                        start=True,
                        stop=False,
                    )
                # Gate-major K-chunk groups (measured best on HW vs the
                # chunk-major-in-arrival-order variants): gate g completes
                # first so the c-chain EW overlaps the i/f/o groups.
                for gi, g in enumerate((3, 0, 1, 2)):
                    for kk in range(8):
                        mm = nc.tensor.matmul(
                            pst[g][:],
                            whh_sb[:, kk, g * 128 : (g + 1) * 128],
                            h_all[:, kk * B : (kk + 1) * B],
                            start=False,
                            stop=(kk == 7),
                        )
                        if gi == 0 and kk == 0:
                            for w in step_waits:
                                _add_dep_helper(
                                    mm.ins,
                                    w.ins,
                                    False,
                                    "first mm after rdma wait",
                                )
                    prev_mm = mm
                    if g == 3:
                        gr = ew.tile([128, B], F32, tag="gr")
                        nc.vector.tensor_scalar_max(gr[:], pst[3][:], 0.0)
                    elif g == 0:
                        sig_i = ew.tile([128, B], F32, tag="sig_i")
                        nc.scalar.activation(sig_i[:], pst[0][:], AF.Sigmoid)
                        ig = ew.tile([128, B], F32, tag="ig")
                        nc.vector.tensor_tensor(ig[:], sig_i[:], gr[:], ALU.mult)
                    elif g == 1:
                        sig_f = ew.tile([128, B], F32, tag="sig_f")
                        nc.scalar.activation(sig_f[:], pst[1][:], AF.Sigmoid)
                        fc = ew.tile([128, B], F32, tag="fc")
                        nc.vector.tensor_tensor(
                            fc[:], sig_f[:], c_prev[:], ALU.mult
                        )
                        c_new = ew.tile([128, B], F32, tag="c")
                        nc.vector.tensor_tensor(c_new[:], fc[:], ig[:], ALU.add)
                        # relu on DVE: keeps the c-chain on one engine so the
                        # tail is sig_f(ACT) -> fc/c_new/rc/hbf all-DVE with a
                        # single cross-engine hop.
                        rc = ew.tile([128, B], F32, tag="rc")
                        nc.vector.tensor_scalar_max(rc[:], c_new[:], 0.0)
                    else:
                        sig_o = ew.tile([128, B], F32, tag="sig_o")
                        nc.scalar.activation(sig_o[:], pst[2][:], AF.Sigmoid)
                # exchange h^T chunks across the 8 cores
                if exchange == "rdma":
                    # write own chunk straight into the landing buffer; the
                    # broadcast carries it to the 7 peers (slot 0 = self is
                    # None: a router-loopback DMA to own SBUF faults the
                    # device).
                    # WAR on the parity buffer (our step-(t-2) broadcast's
                    # deferred SBUF read vs this write) is impossible by
                    # protocol: every peer's step-(t-1) send — which this
                    # step's matmuls waited on — required our step-(t-2)
                    # chunk to have been delivered, i.e. read out of SBUF.
                    hn = hallp.tile([128, 8 * B], BF, tag="hall", name="hn")
                    hw = nc.vector.tensor_tensor(
                        hn[:, bass.ts(pid_dve, B)], sig_o[:], rc[:], ALU.mult
                    )
                    prev_hw = hw
                    evict_pin[0] = hw
                    for rdests in rdest_groups:
                        nc.gpsimd.remote_dma_broadcast(
                            hn[:, bass.ts(pid_sv, B)],
                            hn[:, bass.ts(pid_sv, B)],
                            rsems[t % 2],
                            lsem,
                            rdests=rdests,
                        )
                    trig = nc.gpsimd.trigger_dma(count=None)
                    _add_dep_helper(
                        trig.ins, hw.ins, False, "trigger after own-chunk write"
                    )
                    h_all = hn
                else:
                    hbf = ew.tile([128, B], BF, tag="hbf")
                    hbf_i = nc.vector.tensor_tensor(
                        hbf[:], sig_o[:], rc[:], ALU.mult
                    )
                    evict_pin[0] = hbf_i
                    ci = dram.tile([128, B], BF, tag="ccin")
                    co = dram.tile([128 * NCORES, B], BF, tag="ccout")
                    # bounce-out as two half-width DMAs on the two HWDGE
                    # engines: halves the transfer and runs the ~0.6us issue
                    # cost in parallel, so the collective triggers earlier.
                    nc.sync.dma_start(ci[:, 0 : B // 2], hbf[:, 0 : B // 2])
                    nc.scalar.dma_start(ci[:, B // 2 :], hbf[:, B // 2 :])
                    nc.gpsimd.collective_compute(
                        "AllGather",
                        ALU.bypass,
                        replica_groups=[list(range(NCORES))],
                        ins=[ci.opt()],
                        outs=[co.opt()],
                    )
                    # gather readback: each dma_start costs ~660ns of issue
                    # time on its HWDGE engine and they serialize per engine,
                    # so use 4 two-chunk DMAs split across the two HWDGE
                    # engines (SP + ACT) instead of 8 serial ones on SP.
                    h_all = hallp.tile([128, 8 * B], BF, tag="hall")
                    hv = h_all[:].rearrange("p (r b) -> p r b", r=NCORES)
                    cv = co.rearrange("(r p) b -> p r b", p=128)
                    for eng, r0 in (
                        (nc.sync, 0),
                        (nc.scalar, 4),
                        (nc.sync, 2),
                        (nc.scalar, 6),
                    ):
                        eng.dma_start(
                            hv[:, r0 : r0 + 2, :], cv[:, r0 : r0 + 2, :]
                        )
                c_prev = c_new

                # PE filler: one phase-1 half-unit in this step's exchange
                # window, pinned after the step's matmuls; the next step's
                # rdma wait then pins after the filler (via prev_mm).
                if fgen is not None:
                    fm = next(fgen, None)
                    if fm is not None:
                        if prev_mm is not None:
                            _add_dep_helper(
                                fm.ins, prev_mm.ins, False, "filler after step"
                            )
                        prev_mm = fm

            # ---- phase 3: out = h @ W_o + b_o ----
            step_waits = []
            if exchange == "rdma":
                w = nc.tensor.wait_ge(rsems[(t_steps - 1) % 2], 0)
                deferred_waits.append((w, 14 * ((t_steps + 1) // 2)))
                step_waits.append(w)
                if prev_mm is not None:
                    _add_dep_helper(w.ins, prev_mm.ins, False, "ph3 wait")
            pso = psp.tile([B, O], F32, tag="pA", name="pso")
            mm = nc.tensor.matmul(
                pso[:], ones_sb[:], bo_sb[:], start=True, stop=False
            )
            for w in step_waits:
                _add_dep_helper(mm.ins, w.ins, False, "ph3 mm after wait")
            for kk in range(8):
                nc.tensor.matmul(
                    pso[:],
                    h_all[:, kk * B : (kk + 1) * B],
                    wo_sb[:, kk, :],
                    start=False,
                    stop=(kk == 7),
                )
            out_sb = ew.tile([B, O], F32, tag="osb")
            nc.vector.tensor_copy(out_sb[:], pso[:])
            nc.sync.dma_start(out[:, :], out_sb[:])

    # Patch the real remote-sem thresholds now that Tile has scheduled
    # (placement was pinned with nosync deps during emission).
    for w, val in deferred_waits:
        w.ins.sync_info.on_wait[0].wait_value = val

    nc.compile()
    return nc


def prep_inputs(xs, W_ih, W_hh, b, W_o, b_o):
    """Host-side sharding/layout. Returns in_maps for the 8 cores."""
    xs = np.asarray(xs, dtype=np.float32)
    W_ih = np.asarray(W_ih, dtype=np.float32)
    W_hh = np.asarray(W_hh, dtype=np.float32)
    b = np.asarray(b, dtype=np.float32)
    W_o = np.asarray(W_o, dtype=np.float32)
    b_o = np.asarray(b_o, dtype=np.float32)

    # xs^T in (i, t*64+b) layout, shared by all cores
    xs_t = np.ascontiguousarray(
        xs.transpose(2, 1, 0).reshape(I, T * B)
    ).astype(BF16)
    ident = np.eye(128, dtype=BF16)
    ones = np.ones((1, B), dtype=BF16)
    wo_bf = np.ascontiguousarray(W_o).astype(BF16)
    bo_bf = np.ascontiguousarray(b_o[None, :]).astype(BF16)

    in_maps = []
    for j in range(NCORES):
        # gate columns for core j, permuted to [i, f, o, g] (orig order i,f,g,o)
        cols = np.concatenate(
            [
                np.arange(g * H + j * HSL, g * H + (j + 1) * HSL)
                for g in (0, 1, 3, 2)
            ]
        )
        in_maps.append(
            {
                "xs_t": xs_t,
                "wih": np.ascontiguousarray(W_ih[:, cols]).astype(BF16),
                "whh": np.ascontiguousarray(W_hh[:, cols]).astype(BF16),
                "bias": np.ascontiguousarray(
                    b[cols].reshape(4, HSL).T
                ).astype(np.float32),
                "wo": wo_bf,
                "bo": bo_bf,
                "ident": ident,
                "ones": ones,
            }
        )
    return in_maps


_NC_CACHE = {}
EXCHANGE = "cc"


def _get_nc(t_steps: int = T, exchange: str | None = None):
    import os

    exchange = exchange or os.environ.get("LSTM_EXCHANGE") or EXCHANGE
    interleave = os.environ.get("LSTM_INTERLEAVE", "1") == "1"
    key = (t_steps, exchange, interleave)
    if key not in _NC_CACHE:
        _NC_CACHE[key] = build_program(t_steps, exchange, interleave)
    return _NC_CACHE[key]


def _run(inputs, trace=False):
    nc = _get_nc(T)
    in_maps = prep_inputs(**inputs)
    # The fleet shows occasional transient NRT_EXEC_UNIT_UNRECOVERABLE faults
    # that clear after a short wait; retry a couple of times.
    last_err = None
    for attempt in range(3):
        try:
            res = bass_utils.run_bass_kernel_spmd(
                nc, in_maps, core_ids=list(range(NCORES)), trace=trace
            )
            out = np.asarray(res.results[0]["out"], dtype=np.float32)
            return out, res
        except Exception as e:  # noqa: BLE001 - device-transient errors
            last_err = e
            if attempt < 2:
                import time

                time.sleep(45)
    raise last_err


def kernel(**inputs) -> np.ndarray:
    out, _ = _run(inputs, trace=False)
    return out


def run_traced(**inputs):
    return _run(inputs, trace=True)

